# revision 46
# baseline (speedup 1.0000x reference)
"""Trainium2 Bass kernel for nn_Attention_42657615184259.

Multi-head attention block: x:[8,2048,384] -> qkv proj -> 6-head SDPA
(full softmax) -> out proj -> y:[8,2048,384].

Sharding: data-parallel over batch B=8, one batch element per NeuronCore.

Per-core design (everything in "transposed" space, contraction dims on
SBUF partitions; all matmuls bf16, accumulation/psum fp32):
  1. xT[c,n] built from x (host-cast to bf16) via PE transposes.
  2. qkT[j,n] = qkv_w[:768] @ x.T, stored bf16. v kept in natural layout
     v'[n, h, 0:64] with a ones column at [:, h, 64] so the AV matmul's
     extra output row yields the softmax denominators Z for free.
  3. Per head: scoresT[k,q] = kT.T @ qT -> exp(SCALE*s) on ScalarE
     (PSUM->SBUF, bf16) -> out'[0:65, q] += v'_h.T @ expT over k chunks.
  4. 1/Z per half, reshaped to [128, n/128] for the reciprocal, bounced
     through DRAM with a step-0 partition AP to broadcast across
     partitions, then attnT normalized in place.
  5. proj accumulates all heads in PSUM; + bias on DVE; DMA out.
The emission order interleaves qkT/v'/x-transposes under head-0/1/2's
exp stream so ScalarE (the 25M-exp bottleneck, ~214us busy) saturates
from ~19us on: dependency-free warm-up matmuls lift the PE HAM clock
gate at t~0 and the first four score chunks use 512-wide bites so exp
starts after the first quarter of x. ScalarE and the PE are both
~95-100% busy during the main phase. Best measured 291us/core on TRN2
(run-to-run machine variance ~291-330us).
"""

import os
import numpy as np
from contextlib import ExitStack

DIM = 384
HEADS = 6
DK = 64
N_TOK = 2048
B = 8
N_CORES = 8

_module_cache = {}


def build_module(n_tok=N_TOK, dim=DIM, heads=HEADS, debug=False):
    """Build + compile the per-core Bass module. Returns the Bacc object."""
    import concourse.bass as bass
    import concourse.tile as tile
    from concourse import bacc, mybir
    from concourse.masks import make_identity

    f32 = mybir.dt.float32
    bf16 = mybir.dt.bfloat16
    AF = mybir.ActivationFunctionType
    ALU = mybir.AluOpType

    assert dim % 128 == 0 and n_tok % 2048 == 0 and dim == heads * DK
    CC = dim // 128          # contraction chunks over model dim
    JC = 2 * dim // 128      # q+k row chunks
    NCH = n_tok // 128       # token chunks of 128
    NQ4 = n_tok // 512       # token chunks of 512
    HALF = n_tok // 2
    SCALE = DK ** -0.5

    nc = bacc.Bacc("TRN2", target_bir_lowering=False, debug=debug)

    x_d = nc.dram_tensor("x_b", [n_tok, dim], bf16, kind="ExternalInput").ap()
    qkw_d = nc.dram_tensor("qkw_t", [dim, 2 * dim], bf16, kind="ExternalInput").ap()
    vw_d = nc.dram_tensor("vw_t", [dim, 512], bf16, kind="ExternalInput").ap()
    pw_d = nc.dram_tensor("pw_t", [dim, dim], f32, kind="ExternalInput").ap()
    qkb_d = nc.dram_tensor("qk_b", [2 * dim], f32, kind="ExternalInput").ap()
    vb_d = nc.dram_tensor("v_b", [dim], f32, kind="ExternalInput").ap()
    pb_d = nc.dram_tensor("p_b", [dim], f32, kind="ExternalInput").ap()
    y_d = nc.dram_tensor("y_b", [n_tok, dim], f32, kind="ExternalOutput").ap()

    with tile.TileContext(nc) as tc, ExitStack() as es:
        consts = es.enter_context(tc.tile_pool(name="consts", bufs=1))
        persist = es.enter_context(tc.tile_pool(name="persist", bufs=1))

        # ---- constants / weights (on the gpsimd queue; sync is kept for x) ----
        # PE pre-warm: dependency-free matmuls starting at t~0 lift the HAM
        # clock gate to 2.4GHz before the first real transpose arrives
        junk_sb = consts.tile([128, 128], bf16, tag="junk", name="junk_sb")
        nc.vector.memset(junk_sb, 1.0)
        ident = consts.tile([128, 128], bf16, tag="ident", name="ident")
        make_identity(nc, ident)
        qkwT = []
        vwT = []
        for cc in range(CC):
            t = consts.tile([128, 2 * dim], bf16, tag=f"qkw{cc}", name=f"qkw{cc}")
            nc.gpsimd.dma_start(out=t, in_=qkw_d[cc * 128:(cc + 1) * 128, :])
            qkwT.append(t)
            t = consts.tile([128, 512], bf16, tag=f"vw{cc}", name=f"vw{cc}")
            nc.gpsimd.dma_start(out=t, in_=vw_d[cc * 128:(cc + 1) * 128, :])
            vwT.append(t)
        qkb = []
        for jc in range(JC):
            t = consts.tile([128, 1], f32, tag=f"qkb{jc}", name=f"qkb{jc}")
            nc.gpsimd.dma_start(out=t, in_=qkb_d[jc * 128:(jc + 1) * 128])
            qkb.append(t)
        pwT = []
        for h in range(heads):
            t = consts.tile([64, dim], bf16, tag=f"pw{h}", name=f"pw{h}")
            nc.gpsimd.dma_start(out=t, in_=pw_d[h * 64:(h + 1) * 64, :])
            pwT.append(t)
        # free-axis biases broadcast across partitions via step-0 DMA
        vb_bc = consts.tile([128, dim], f32, tag="vb", name="vb")
        nc.gpsimd.dma_start(
            out=vb_bc,
            in_=bass.AP(tensor=vb_d.tensor, offset=vb_d.offset,
                        ap=[[0, 128], *vb_d.ap]),
        )
        pb_bc = consts.tile([128, dim], f32, tag="pb", name="pb")
        nc.gpsimd.dma_start(
            out=pb_bc,
            in_=bass.AP(tensor=pb_d.tensor, offset=pb_d.offset,
                        ap=[[0, 128], *pb_d.ap]),
        )
        # persistent activations
        qkT = [persist.tile([128, n_tok], bf16, tag=f"qkT{jc}", name=f"qkT{jc}") for jc in range(JC)]
        vp = [persist.tile([128, heads, 65], bf16, tag=f"vp{ni}", name=f"vp{ni}") for ni in range(NCH)]

        attnT = [persist.tile([64, n_tok], bf16, tag=f"attnT{h}", name=f"attnT{h}") for h in range(heads)]

        def qk_slice(row0, col0, ncols):
            """[64, ncols] slice of the qkT row space at row row0 (64-aligned)."""
            ti, po = divmod(row0, 128)
            return qkT[ti][po:po + 64, col0:col0 + ncols]

        # ---- phases B+C, finely interleaved so ACT saturates early ----
        # PSUM budget at any emission point stays at 8 banks:
        #   sps(2x2) + avp(2x1) + bps(2x1, shared by x-transposes and qkT)
        es_bc = es.enter_context(ExitStack())
        sps = es_bc.enter_context(tc.tile_pool(name="sps", bufs=2, space="PSUM"))
        expp = es_bc.enter_context(tc.tile_pool(name="expp", bufs=1))
        zstp = es_bc.enter_context(tc.tile_pool(name="zst", bufs=2))
        zdp = es_bc.enter_context(tc.tile_pool(name="zdram", bufs=2, space="DRAM"))
        rbp = es_bc.enter_context(tc.tile_pool(name="rbp", bufs=2))
        avps = es_bc.enter_context(tc.tile_pool(name="avp", bufs=2, space="PSUM"))
        xtp = tc.alloc_tile_pool(name="xt", bufs=1)
        xTt = xtp.tile([128, CC, n_tok], bf16, tag="xTt", name="xTt")
        xT = [xTt[:, cc, :] for cc in range(CC)]
        bps = tc.alloc_tile_pool(name="bps", bufs=2, space="PSUM")
        xinp = tc.alloc_tile_pool(name="xin", bufs=3)

        def emit_warmup():
            for _ in range(12):
                jp = bps.tile([128, 128], f32, tag="bps", name="jp")
                nc.tensor.matmul(jp, lhsT=junk_sb, rhs=junk_sb,
                                 start=True, stop=True)

        def emit_xchunk(ni):
            xin = xinp.tile([128, dim], bf16, tag="xin", name="xin")
            nc.sync.dma_start(xin, x_d[ni * 128:(ni + 1) * 128, :])
            pt = bps.tile([128, CC, 128], bf16, tag="bps", name="pt")
            for cc in range(CC):
                nc.tensor.transpose(
                    pt[:, cc, :], xin[:, cc * 128:(cc + 1) * 128], ident)
            nc.vector.tensor_copy(xTt[:, :, ni * 128:(ni + 1) * 128], pt)

        def emit_qkT_chunk(jc, q4):
            ps = bps.tile([128, 512], f32, tag="bps", name="qkps")
            for cc in range(CC):
                nc.tensor.matmul(
                    ps,
                    lhsT=qkwT[cc][:, jc * 128:(jc + 1) * 128],
                    rhs=xT[cc][:, q4 * 512:(q4 + 1) * 512],
                    start=(cc == 0), stop=(cc == CC - 1),
                )
            nc.vector.tensor_scalar_add(
                qkT[jc][:, q4 * 512:(q4 + 1) * 512], ps, qkb[jc])

        def emit_v_chunk(ni):
            ps = avps.tile([128, 512], f32, tag="av", name="vps")
            for cc in range(CC):
                nc.tensor.matmul(
                    ps,
                    lhsT=xT[cc][:, ni * 128:(ni + 1) * 128],
                    rhs=vwT[cc],
                    start=(cc == 0), stop=(cc == CC - 1),
                )
            nc.vector.tensor_tensor(
                vp[ni][:, :, 0:64],
                ps[:, 0:dim].rearrange("p (h d) -> p h d", h=heads),
                vb_bc.rearrange("p (h d) -> p h d", h=heads),
                ALU.add,
            )
            nc.vector.memset(vp[ni][:, :, 64:65], 1.0)

        def emit_scores_kc(h, half, kc, ets, pool=None, pfx="e"):
            q0 = half * HALF
            sp = sps.tile([128, 1024], f32, tag="sp", name="sp")
            for qs in range(2):
                nc.tensor.matmul(
                    sp[:, qs * 512:(qs + 1) * 512],
                    lhsT=qk_slice(dim + h * 64, kc * 128, 128),
                    rhs=qk_slice(h * 64, q0 + qs * 512, 512),
                    start=True, stop=True,
                )
            et = (pool or expp).tile([128, 1024], bf16,
                                     tag=f"{pfx}{kc}", name=f"{pfx}{kc}")
            nc.scalar.activation(et, sp, AF.Exp, scale=SCALE)
            ets.append(et)

        def emit_scores(h, half):
            ets = []
            for kc in range(NCH):
                emit_scores_kc(h, half, kc, ets)
            return ets

        def emit_av(h, half, ets, zstage):
            for qs in range(2):
                qc = half * 2 + qs
                av = avps.tile([65, 512], f32, tag="av", name="av")
                for kc in range(NCH):
                    nc.tensor.matmul(
                        av,
                        lhsT=vp[kc][:, h, :],
                        rhs=ets[kc][:, qs * 512:(qs + 1) * 512],
                        start=(kc == 0), stop=(kc == NCH - 1),
                    )
                nc.vector.tensor_copy(
                    attnT[h][:, qc * 512:(qc + 1) * 512], av[0:64, :])
                nc.vector.tensor_copy(
                    zstage[64:65, qc * 512:(qc + 1) * 512], av[64:65, :])

        def emit_pe_warm(dep_ap, nf):
            # tiny matmul reading `dep_ap` -- keeps the PE HAM window warm
            # across the final norm chain so proj doesn't start down-clocked
            p = dep_ap.partition_size()
            jt = avps.tile([1, nf], f32, tag="av", name="junkt")
            nc.tensor.matmul(jt, lhsT=dep_ap[0:p, 0:1], rhs=dep_ap[0:p, 0:nf],
                             start=True, stop=True)

        def emit_norm_half(h, zstage, half, warm=False):
            # 1/Z for this half's columns, bounce through DRAM to broadcast
            # across partitions, then normalize attnT in place. The
            # reciprocal runs on a [128, HALF/128] reshape (a [1, n] row
            # would be ~16us).
            c0 = half * HALF
            zcol = zstp.tile([128, HALF // 128], f32, tag="zcol", name="zcol")
            nc.sync.dma_start(zcol, zstage[64:65, c0:c0 + HALF])
            nc.vector.reciprocal(zcol, zcol)
            zd = zdp.tile([1, HALF], f32, tag="zd", name="zd")
            nc.sync.dma_start(zd, zcol)
            rb = rbp.tile([64, HALF], f32, tag="rb", name="rb")
            if warm:
                emit_pe_warm(zcol, HALF // 128)
            nc.gpsimd.dma_start(
                out=rb,
                in_=bass.AP(tensor=zd.tensor, offset=zd.offset,
                            ap=[[0, 64], zd.ap[-1]]),
            )
            if warm:
                emit_pe_warm(rb, 512)
                emit_pe_warm(rb[:, 512:], 512)
            for qs in range(HALF // 512):
                nc.vector.tensor_tensor(
                    attnT[h][:, c0 + qs * 512:c0 + (qs + 1) * 512],
                    attnT[h][:, c0 + qs * 512:c0 + (qs + 1) * 512],
                    rb[:, qs * 512:(qs + 1) * 512], ALU.mult)

        def emit_head(h, extras=()):
            """One head; `extras` are (kc_index, closure) emitted inside the
            half-0 score loop to soak spare PE cycles under the exp stream."""
            extras = dict(extras)
            zstage = zstp.tile([65, n_tok], f32, tag="zst", name="zst")
            last = h == heads - 1
            for half in range(2):
                ets = []
                for kc in range(NCH):
                    emit_scores_kc(h, half, kc, ets)
                    fn = extras.pop((half, kc), None)
                    if fn is not None:
                        fn()
                emit_av(h, half, ets, zstage)
                emit_norm_half(h, zstage, half, warm=(last and half == 1))
            for fn in extras.values():
                fn()

        if NQ4 == 4 and heads == 6:
            # Pipelined startup: per 512-column group load/transpose x,
            # produce that group's qkT columns for head-0's q/k row-chunks,
            # and start head-0 scores as soon as their inputs exist. v' and
            # the remaining qkT chunks ride in PE slack under the exp stream.
            zstage0 = zstp.tile([65, n_tok], f32, tag="zst", name="zstage0")
            emit_warmup()
            ets0 = []

            def emit_scores_512(kc, qs):
                sp = sps.tile([128, 512], f32, tag="sp", name="sp")
                nc.tensor.matmul(
                    sp,
                    lhsT=qk_slice(dim, kc * 128, 128),
                    rhs=qk_slice(0, qs * 512, 512),
                    start=True, stop=True,
                )
                if qs == 0:
                    et = expp.tile([128, 1024], bf16, tag=f"e{kc}", name=f"e{kc}")
                    ets0.append(et)
                nc.scalar.activation(ets0[kc][:, qs * 512:(qs + 1) * 512],
                                     sp, AF.Exp, scale=SCALE)

            for q4 in range(4):
                for ni in range(4 * q4, 4 * q4 + 4):
                    emit_xchunk(ni)
                emit_qkT_chunk(0, q4)
                emit_qkT_chunk(JC // 2, q4)
                if q4 == 0:
                    for kc in range(0, 4):
                        emit_scores_512(kc, 0)
                elif q4 == 1:
                    for kc in range(0, 4):
                        emit_scores_512(kc, 1)
                    for kc in range(4, 8):
                        emit_scores_kc(0, 0, kc, ets0)
                    for ni in range(0, 4):
                        emit_v_chunk(ni)
                elif q4 == 2:
                    for kc in range(8, 12):
                        emit_scores_kc(0, 0, kc, ets0)
                    for ni in range(4, 10):
                        emit_v_chunk(ni)
                elif q4 == 3:
                    for kc in range(12, 16):
                        emit_scores_kc(0, 0, kc, ets0)
                    for ni in range(10, 16):
                        emit_v_chunk(ni)
            # h0 half1 runs from its own short-lived exp tile set so its
            # exp stream does not wait on AV(0,0)'s reads of the half-0 set
            expp2 = tc.alloc_tile_pool(name="expp2", bufs=1)
            ets1 = []
            for kc in range(0, 9):
                emit_scores_kc(0, 1, kc, ets1, pool=expp2, pfx="f")
            emit_av(0, 0, ets0, zstage0)
            emit_norm_half(0, zstage0, 0)
            for kc in range(9, NCH):
                emit_scores_kc(0, 1, kc, ets1, pool=expp2, pfx="f")
            emit_av(0, 1, ets1, zstage0)
            emit_norm_half(0, zstage0, 1)
            expp2.release()
            # remaining qkT chunks: one j-chunk (4 q4-groups) per half
            jc_sched = {
                1: (((0, 1), lambda: emit_qkT_chunk(4, 0)),
                    ((0, 5), lambda: emit_qkT_chunk(4, 1)),
                    ((0, 9), lambda: emit_qkT_chunk(4, 2)),
                    ((0, 13), lambda: emit_qkT_chunk(4, 3)),
                    ((0, 3), lambda: emit_qkT_chunk(1, 0)),
                    ((0, 11), lambda: emit_qkT_chunk(1, 1)),
                    ((1, 3), lambda: emit_qkT_chunk(1, 2)),
                    ((1, 11), lambda: emit_qkT_chunk(1, 3)),
                    ((1, 1), lambda: emit_qkT_chunk(2, 0)),
                    ((1, 5), lambda: emit_qkT_chunk(2, 1)),
                    ((1, 9), lambda: emit_qkT_chunk(2, 2)),
                    ((1, 13), lambda: emit_qkT_chunk(2, 3))),
                2: (((0, 1), lambda: emit_qkT_chunk(5, 0)),
                    ((0, 5), lambda: emit_qkT_chunk(5, 1)),
                    ((0, 9), lambda: emit_qkT_chunk(5, 2)),
                    ((0, 13), lambda: emit_qkT_chunk(5, 3))),
            }
            emit_head(1, jc_sched[1])
            emit_head(2, jc_sched[2])
            xinp.release()
            bps.release()
            xtp.release()
            first_rest = 3
        else:
            # simple sequential fallback (small configs / sim)
            for ni in range(NCH):
                emit_xchunk(ni)
            for jc in range(JC):
                for q4 in range(NQ4):
                    emit_qkT_chunk(jc, q4)
            for ni in range(NCH):
                emit_v_chunk(ni)
            xinp.release()
            bps.release()
            xtp.release()
            first_rest = 0
        for h in range(first_rest, heads):
            emit_head(h)
        es_bc.close()  # release attention pools before proj

        # ---- phase D: projection ----
        with tc.tile_pool(name="projp", bufs=2, space="PSUM") as projp, \
             tc.tile_pool(name="ypool", bufs=3) as ypool:
            for ni in range(NCH):
                yp = projp.tile([128, dim], f32, tag="yp", name="yp")
                for h in range(heads):
                    nc.tensor.matmul(
                        yp,
                        lhsT=attnT[h][:, ni * 128:(ni + 1) * 128],
                        rhs=pwT[h],
                        start=(h == 0), stop=(h == heads - 1),
                    )
                yout = ypool.tile([128, dim], f32, tag="yout", name="yout")
                nc.vector.tensor_add(yout, yp, pb_bc)
                eng = nc.sync if ni % 2 == 0 else nc.scalar
                eng.dma_start(y_d[ni * 128:(ni + 1) * 128, :], yout)

    nc.compile()
    return nc


def make_in_maps(x, qkv_w, qkv_b, proj_w, proj_b, n_cores=N_CORES):
    """Host-side shard prep: per-core input dicts (weights host-transposed)."""
    x = np.asarray(x, dtype=np.float32)
    qkv_w = np.asarray(qkv_w, dtype=np.float32)
    qkv_b = np.asarray(qkv_b, dtype=np.float32)
    proj_w = np.asarray(proj_w, dtype=np.float32)
    proj_b = np.asarray(proj_b, dtype=np.float32)
    dim = x.shape[-1]
    import ml_dtypes
    bf16 = ml_dtypes.bfloat16
    shared = {
        "qkw_t": np.ascontiguousarray(qkv_w[:2 * dim].T.astype(bf16)),
        "vw_t": np.ascontiguousarray(
            np.pad(qkv_w[2 * dim:3 * dim].T, ((0, 0), (0, 512 - dim))).astype(bf16)),
        "pw_t": np.ascontiguousarray(proj_w.T),
        "qk_b": np.ascontiguousarray(qkv_b[:2 * dim]),
        "v_b": np.ascontiguousarray(qkv_b[2 * dim:3 * dim]),
        "p_b": np.ascontiguousarray(proj_b),
    }
    return [
        {"x_b": np.ascontiguousarray(x[i].astype(bf16)), **shared}
        for i in range(x.shape[0])
    ]


def run_on_hw(nc, in_maps, trace=False, trace_cores=None):
    from concourse import bass_utils
    return bass_utils.run_bass_kernel_spmd(
        nc, in_maps, core_ids=list(range(len(in_maps))),
        trace=trace, trace_cores=trace_cores,
    )


def kernel(x, qkv_w, qkv_b, proj_w, proj_b):
    key = (N_TOK, DIM, HEADS)
    if key not in _module_cache:
        _module_cache[key] = build_module(*key)
    nc = _module_cache[key]
    in_maps = make_in_maps(x, qkv_w, qkv_b, proj_w, proj_b)
    res = run_on_hw(nc, in_maps)
    y = np.stack([res.results[i]["y_b"] for i in range(len(in_maps))])
    return y.astype(np.float32)


if __name__ == "__main__":
    import reference
    inputs = reference.setup_inputs()
    out = kernel(**{k: np.asarray(v) for k, v in inputs.items()})
    print("out", out.shape, out.dtype)
